# revision 1
# baseline (speedup 1.0000x reference)
"""Trainium2 Bass kernel for nn_MEModule (gnn_message_passing).

Math per edge e (reference):
    h_emb = [h[idx_s[e]], h[idx_t[e]]]                 # [24]
    a     = h_emb @ w1cat + b1cat                      # [72]  (w1cat[d,(m,f)] = w1[m,d,f])
    g     = h_emb @ w2cat + b2cat                      # [72]
    glu   = a * sigmoid(g)                             # [72]
    stk   = glu * rbf3          (rbf3[(m,d)] = rbf[d]) # [72]
    out   = stk @ wl + bl                              # [128]

Device layout ("T-layout"): edges on the free dim, features on partitions.
Host pre-gathers h_emb, pre-transposes, and interleaves with rbf into one
stream hr = [h_embT; rbf_T] of shape [48, E]; output is produced as
[128, E] and de-transposed on the host.  8-way edge sharding; no
collectives.  All weights travel in one packed [128, 347] tensor so every
matmul depends on a single weight-DMA semaphore.

Per 500-edge chunk on device:
    a_ps   = w1cat.T @ h_embT          (PE, PSUM [72,500])
    g_ps   = w2cat.T @ h_embT          (PE)
    r_ps   = brep.T  @ rbf_T           (PE; brep = [I24 I24 I24] replicates rbf)
    sig    = sigmoid(g_ps + b2cat)     (ACT, bias = per-partition AP)
    glu    = (a_ps + b1cat) * sig      (DVE scalar_tensor_tensor)
    stk    = glu * r_ps                (DVE tensor_mul)
    o_ps   = wl.T @ stk                (PE, PSUM [128,500])
    out    = o_ps + blcat              (ACT Identity w/ bias -> SBUF, DMA out)

Engine budget per core (250k edges), estimated from TRN2 specs:
    DMA  ~536us  (in 2x24MB + out 128MB @ ~332GB/s)  <- binding (memory regime)
    DVE  ~521us  (2 ops/chunk @ 0.96GHz, free-dim 500)
    ACT  ~420us  (sigmoid + biased PSUM->SBUF copy @ 1.2GHz)
    PE   ~417us  (4 matmul passes, N=500 @ 2.4GHz, static weights)
PSUM: 4 tags x 2 bufs x 1 bank = all 8 banks.  Measured end-to-end:
rel err 5.5e-7 vs fp32 reference; wall-clock per dispatch ~88.7ms under
axon PJRT (RPC-dominated; NTFF profiling hook unavailable in this env).
Known next levers: merge adjacent-chunk ACT copies (fewer fixed 172c PSUM
access overheads), split the stk mul ACT/DVE to rebalance, deeper sb bufs.
"""

import numpy as np

N_CORES = 8
E_TOTAL = 2_000_000
EMB = 12
D = 24            # 2*EMB
HR = 2 * D        # 48: h_embT rows + rbf_T rows
KF = 72           # NUM_MODULES * D
OUT = 128
SUPER = 5000      # edges per DMA supertile
CHUNK = 500       # edges per PSUM chunk (matmul N, <=512 fp32)

# packed-weights column layout ([128, WP_F] tensor)
W1_C, W2_C, BR_C, WL_C = 0, 72, 144, 216
B1_C, B2_C, BL_C = 344, 345, 346
WP_F = 347


def build_nc(e_shard: int, super_: int = SUPER, chunk: int = CHUNK):
    from contextlib import ExitStack

    import concourse.tile as tile
    from concourse import bacc, mybir

    f32 = mybir.dt.float32
    assert e_shard % super_ == 0 and super_ % chunk == 0
    n_super = e_shard // super_
    n_chunk = super_ // chunk

    try:
        from concourse._compat import get_trn_type
        trn = get_trn_type() or "TRN2"
    except Exception:
        trn = "TRN2"
    nc = bacc.Bacc(trn, target_bir_lowering=False, debug=False)
    hr = nc.declare_dram_parameter("hr", [D, 2 * e_shard], f32, isOutput=False)
    wpk = nc.declare_dram_parameter("wpack", [OUT, WP_F], f32, isOutput=False)
    outT = nc.declare_dram_parameter("outT", [OUT, e_shard], f32, isOutput=True)

    with ExitStack() as ctx:
        tc = ctx.enter_context(tile.TileContext(nc))
        wpool = ctx.enter_context(tc.tile_pool(name="weights", bufs=1))
        sb = ctx.enter_context(tc.tile_pool(name="sbuf", bufs=2))
        vb = ctx.enter_context(tc.tile_pool(name="vecbuf", bufs=2))
        ps = ctx.enter_context(tc.tile_pool(name="psum", bufs=2, space="PSUM"))

        wp = wpool.tile([OUT, WP_F], f32, tag="wp")
        nc.sync.dma_start(out=wp[:], in_=wpk[:])
        w1_t = wp[0:D, W1_C : W1_C + KF]
        w2_t = wp[0:D, W2_C : W2_C + KF]
        br_t = wp[0:D, BR_C : BR_C + KF]
        wl_t = wp[0:KF, WL_C : WL_C + OUT]
        b1_t = wp[0:KF, B1_C : B1_C + 1]
        b2_t = wp[0:KF, B2_C : B2_C + 1]
        bl_t = wp[0:OUT, BL_C : BL_C + 1]

        for st in range(n_super):
            s0 = st * super_
            hrt = sb.tile([D, 2 * super_], f32, tag="hrt")
            ot = sb.tile([OUT, super_], f32, tag="ot")
            nc.sync.dma_start(out=hrt[:], in_=hr[:, 2 * s0 : 2 * s0 + 2 * super_])
            for c in range(n_chunk):
                sl = slice(c * chunk, (c + 1) * chunk)
                ht = hrt[0:D, c * chunk : (c + 1) * chunk]
                rt = hrt[0:D, super_ + c * chunk : super_ + (c + 1) * chunk]
                a_ps = ps.tile([KF, chunk], f32, tag="a")
                g_ps = ps.tile([KF, chunk], f32, tag="g")
                r_ps = ps.tile([KF, chunk], f32, tag="r")
                o_ps = ps.tile([OUT, chunk], f32, tag="o")
                nc.tensor.matmul(out=a_ps[:], lhsT=w1_t, rhs=ht,
                                 start=True, stop=True)
                nc.tensor.matmul(out=g_ps[:], lhsT=w2_t, rhs=ht,
                                 start=True, stop=True)
                nc.tensor.matmul(out=r_ps[:], lhsT=br_t, rhs=rt,
                                 start=True, stop=True)
                sig = vb.tile([KF, chunk], f32, tag="sig")
                nc.scalar.activation(out=sig[:], in_=g_ps[:],
                                     func=mybir.ActivationFunctionType.Sigmoid,
                                     bias=b2_t, scale=1.0)
                glu = vb.tile([KF, chunk], f32, tag="glu")
                nc.vector.scalar_tensor_tensor(out=glu[:], in0=a_ps[:],
                                               scalar=b1_t, in1=sig[:],
                                               op0=mybir.AluOpType.add,
                                               op1=mybir.AluOpType.mult)
                stk = vb.tile([KF, chunk], f32, tag="stk")
                nc.vector.tensor_mul(out=stk[:], in0=glu[:], in1=r_ps[:])
                nc.tensor.matmul(out=o_ps[:], lhsT=wl_t, rhs=stk[:],
                                 start=True, stop=True)
                nc.scalar.activation(out=ot[:, sl], in_=o_ps[:],
                                     func=mybir.ActivationFunctionType.Identity,
                                     bias=bl_t, scale=1.0)
            nc.sync.dma_start(out=outT[:, s0 : s0 + super_], in_=ot[:])
    nc.compile()
    return nc


def pack_weights(w1, b1, w2, b2, wl, bl):
    wp = np.zeros((OUT, WP_F), dtype=np.float32)
    w1cat = np.asarray(w1, np.float32).transpose(1, 0, 2).reshape(D, KF)
    w2cat = np.asarray(w2, np.float32).transpose(1, 0, 2).reshape(D, KF)
    brep = np.concatenate([np.eye(D, dtype=np.float32)] * 3, axis=1)
    wp[0:D, W1_C : W1_C + KF] = w1cat
    wp[0:D, W2_C : W2_C + KF] = w2cat
    wp[0:D, BR_C : BR_C + KF] = brep
    wp[0:KF, WL_C : WL_C + OUT] = np.asarray(wl, np.float32)
    wp[0:KF, B1_C] = np.asarray(b1, np.float32).reshape(KF)
    wp[0:KF, B2_C] = np.asarray(b2, np.float32).reshape(KF)
    wp[0:OUT, BL_C] = np.asarray(bl, np.float32).reshape(OUT)
    return wp


def prep_inputs(rbf, h, idx_s, idx_t, w1, b1, w2, b2, wl, bl,
                e_total=E_TOTAL, n_cores=N_CORES):
    """Host-side marshaling: gather, transpose, shard."""
    rbf = np.asarray(rbf, dtype=np.float32)
    h = np.asarray(h, dtype=np.float32)
    idx_s = np.asarray(idx_s).astype(np.int64)
    idx_t = np.asarray(idx_t).astype(np.int64)
    ec = e_total // n_cores

    # Per-supertile interleave: hr[:, 2*s0 : 2*s0+S] = h_embT block,
    # hr[:, 2*s0+S : 2*s0+2S] = rbf_T block, so the device loads one
    # [24, 2S] tile per supertile with both operands at base partition 0.
    hembT = np.empty((D, e_total), dtype=np.float32)
    hembT[0:EMB, :] = h[idx_s].T
    hembT[EMB:D, :] = h[idx_t].T
    rbfT = rbf.T
    n_super = ec // SUPER
    wp = pack_weights(w1, b1, w2, b2, wl, bl)
    in_maps = []
    for i in range(n_cores):
        s = slice(i * ec, (i + 1) * ec)
        hb = hembT[:, s].reshape(D, n_super, SUPER)
        rb = rbfT[:, s].reshape(D, n_super, SUPER)
        hr = np.ascontiguousarray(
            np.stack([hb, rb], axis=2).reshape(D, 2 * ec))
        in_maps.append({"hr": hr, "wpack": wp})
    return in_maps


def build_exec(nc, in_maps):
    """Mirror bass2jax.run_bass_via_pjrt but stage inputs on device once and
    return (fn, dev_args, assemble) so callers can time pure execution."""
    import jax
    import jax.numpy as jnp
    from jax.sharding import Mesh, PartitionSpec, NamedSharding
    from jax.experimental.shard_map import shard_map
    import concourse.mybir as mybir
    from concourse.bass2jax import (_bass_exec_p, install_neuronx_cc_hook,
                                    partition_id_tensor)

    install_neuronx_cc_hook()
    n_cores = len(in_maps)
    in_names, out_names, out_avals = [], [], []
    partition_name = (nc.partition_id_tensor.name
                      if nc.partition_id_tensor else None)
    for alloc in nc.m.functions[0].allocations:
        if not isinstance(alloc, mybir.MemoryLocationSet):
            continue
        name = alloc.memorylocations[0].name
        if alloc.kind == "ExternalInput":
            if name != partition_name:
                in_names.append(name)
        elif alloc.kind == "ExternalOutput":
            out_names.append(name)
            out_avals.append(jax.core.ShapedArray(
                tuple(alloc.tensor_shape), mybir.dt.np(alloc.dtype)))
    n_params = len(in_names)
    all_in_names = list(in_names) + list(out_names)
    if partition_name is not None:
        all_in_names.append(partition_name)

    def _body(*args):
        operands = list(args)
        if partition_name is not None:
            operands.append(partition_id_tensor())
        return tuple(_bass_exec_p.bind(
            *operands,
            out_avals=tuple(out_avals),
            in_names=tuple(all_in_names),
            out_names=tuple(out_names),
            lowering_input_output_aliases=(),
            sim_require_finite=True,
            sim_require_nnan=True,
            nc=nc,
        ))

    devices = jax.devices()[:n_cores]
    mesh = Mesh(np.asarray(devices), ("core",))
    n_outs = len(out_names)
    in_specs = (PartitionSpec("core"),) * (n_params + n_outs)
    out_specs = (PartitionSpec("core"),) * n_outs
    fn = jax.jit(shard_map(_body, mesh=mesh, in_specs=in_specs,
                           out_specs=out_specs, check_rep=False),
                 keep_unused=True)
    sh = NamedSharding(mesh, PartitionSpec("core"))
    dev_args = []
    for i, name in enumerate(in_names):
        cat = np.concatenate([np.asarray(m[name]) for m in in_maps], axis=0)
        dev_args.append(jax.device_put(cat, sh))
    for av in out_avals:
        z = jnp.zeros((n_cores * av.shape[0], *av.shape[1:]), av.dtype)
        dev_args.append(jax.device_put(z, sh))

    def assemble(out_arrs):
        res = []
        for c in range(n_cores):
            res.append({name: np.asarray(out_arrs[i]).reshape(
                n_cores, *out_avals[i].shape)[c]
                for i, name in enumerate(out_names)})
        return res

    return fn, dev_args, assemble


def run(rbf, h, idx_s, idx_t, w1, b1, w2, b2, wl, bl, time_iters=0):
    import time as _time

    e_total = rbf.shape[0]
    ec = e_total // N_CORES
    in_maps = prep_inputs(rbf, h, idx_s, idx_t, w1, b1, w2, b2, wl, bl,
                          e_total=e_total)
    nc = build_nc(ec)
    fn, dev_args, assemble = build_exec(nc, in_maps)
    out_arrs = fn(*dev_args)  # compile + first run
    import jax
    jax.block_until_ready(out_arrs)
    times = []
    for _ in range(time_iters):
        t0 = _time.perf_counter()
        jax.block_until_ready(fn(*dev_args))
        times.append(_time.perf_counter() - t0)
    results = assemble(out_arrs)
    out = np.empty((e_total, OUT), dtype=np.float32)
    for i in range(N_CORES):
        out[i * ec : (i + 1) * ec] = results[i]["outT"].T
    return out, times


def kernel(rbf, h, idx_s, idx_t, w1, b1, w2, b2, wl, bl):
    """Full-input entry point: shard across 8 cores, run the Bass kernel
    via run_bass_kernel_spmd, gather back to the full [E, 128] output."""
    from concourse.bass_utils import run_bass_kernel_spmd

    e_total = rbf.shape[0]
    ec = e_total // N_CORES
    in_maps = prep_inputs(rbf, h, idx_s, idx_t, w1, b1, w2, b2, wl, bl,
                          e_total=e_total)
    nc = build_nc(ec)
    res = run_bass_kernel_spmd(nc, in_maps, list(range(N_CORES)))
    out = np.empty((e_total, OUT), dtype=np.float32)
    for i in range(N_CORES):
        out[i * ec : (i + 1) * ec] = res.results[i]["outT"].T
    return out



# revision 2
# speedup vs baseline: 1.3017x; 1.3017x over previous
"""Trainium2 Bass kernel for nn_MEModule (gnn_message_passing).

Math per edge e (reference):
    h_emb = [h[idx_s[e]], h[idx_t[e]]]                 # [24]
    a     = h_emb @ w1cat + b1cat                      # [72]  (w1cat[d,(m,f)] = w1[m,d,f])
    g     = h_emb @ w2cat + b2cat                      # [72]
    glu   = a * sigmoid(g)                             # [72]
    stk   = glu * rbf3          (rbf3[(m,d)] = rbf[d]) # [72]
    out   = stk @ wl + bl                              # [128]

Device layout ("T-layout"): edges on the free dim, features on partitions.
Host pre-gathers h_emb -> hs [24, E] bf16 and pre-replicates rbf3 ->
rb [72, E] bf16 (separate tensors: SBUF operands spanning >32 partitions
must start at partition 0).  Output is produced as outT [128, E] bf16 and
de-transposed + f32-cast on the host.  8-way edge sharding; no collectives.

Per 500-edge chunk on device:
    a_ps = w1cat.T @ ht          (PE, bf16 inputs, PSUM f32 [72,500])
    g_ps = w2cat.T @ ht          (PE)
    sig  = sigmoid(g_ps + b2)    (ACT, bias AP, out SBUF bf16)
    glu  = (a_ps + b1) * sig     (DVE scalar_tensor_tensor)
    stk  = glu * rb_chunk        (DVE tensor_mul, all-SBUF bf16 -> 2x mode)
    o_ps = wl.T @ stk            (PE, PSUM [128,500])
    out  = o_ps + bl             (3 of 4 chunks: ACT Identity+bias;
                                  1 of 4: DVE tensor_scalar_add -> SBUF bf16)

Why this shape (evidence from the TimelineSim cost model, validated on HW):
  * fp32 matmuls cost 4 cycles/row on TRN2 -> bf16 operands are 4x faster
    on the PE; the v1 all-f32 kernel was PE-bound at 99% (1.68ms/core sim).
  * rbf3 host-replication kills v1's PE replication matmul and makes the
    stk multiply an all-SBUF 16-bit DVE op (2x mode).
  * bf16 stream + bf16 output halves DMA traffic (112MB/core).
  * PSUM->SBUF copies split 3:1 between ACT and DVE balances the two
    elementwise engines (GPSIMD cannot read PSUM, so Pool can't help).
  * Simulated 583us/core (v1: 1701us); engines DVE 86% / ACT 72% /
    PE 56% / DMA 55%.
HW (axon PJRT, 8 cores): rel err 6.1e-3 vs fp32 reference; steady-state
pipelined dispatch ~1.5-3.5ms/call (single-dispatch wall latency is
RTT-dominated at ~60-90ms on this tunnel).
"""

import numpy as np

N_CORES = 8
E_TOTAL = 2_000_000
EMB = 12
D = 24            # 2*EMB
KF = 72           # NUM_MODULES * D
HS_P = 24         # h_embT stream rows
RB_P = 72         # rbf3T stream rows
OUT = 128
SUPER = 5000      # edges per DMA supertile
CHUNK = 500       # edges per PSUM chunk (<=512 fp32 per bank)
STST_ENG = "D"    # glu op engine pattern (D=DVE)
COPY_ENG = "AAAD" # output-copy engine pattern (A=ACT, D=DVE)

# packed bf16 weight columns ([128, WB_F]); f32 biases ([128, 3])
W1_C, W2_C, WL_C = 0, 72, 144
WB_F = 272


def build_nc(e_shard: int, super_: int = SUPER, chunk: int = CHUNK,
             stst_eng: str = STST_ENG, copy_eng: str = COPY_ENG):
    from contextlib import ExitStack

    import concourse.tile as tile
    from concourse import bacc, mybir

    f32 = mybir.dt.float32
    bf16 = mybir.dt.bfloat16
    assert e_shard % super_ == 0 and super_ % chunk == 0
    n_super = e_shard // super_
    n_chunk = super_ // chunk

    try:
        from concourse._compat import get_trn_type
        trn = get_trn_type() or "TRN2"
    except Exception:
        trn = "TRN2"
    nc = bacc.Bacc(trn, target_bir_lowering=False, debug=False)
    hs = nc.declare_dram_parameter("hs", [HS_P, e_shard], bf16, isOutput=False)
    rb = nc.declare_dram_parameter("rb", [RB_P, e_shard], bf16, isOutput=False)
    wb = nc.declare_dram_parameter("wb", [OUT, WB_F], bf16, isOutput=False)
    wf = nc.declare_dram_parameter("wf", [OUT, 3], f32, isOutput=False)
    outT = nc.declare_dram_parameter("outT", [OUT, e_shard], bf16, isOutput=True)

    with ExitStack() as ctx:
        tc = ctx.enter_context(tile.TileContext(nc))
        wpool = ctx.enter_context(tc.tile_pool(name="weights", bufs=1))
        sb = ctx.enter_context(tc.tile_pool(name="sbuf", bufs=2))
        vb = ctx.enter_context(tc.tile_pool(name="vecbuf", bufs=2))
        ps = ctx.enter_context(tc.tile_pool(name="psum", bufs=2, space="PSUM"))

        wbt = wpool.tile([OUT, WB_F], bf16, tag="wb")
        wft = wpool.tile([OUT, 3], f32, tag="wf")
        nc.sync.dma_start(out=wbt[:], in_=wb[:])
        nc.sync.dma_start(out=wft[:], in_=wf[:])
        w1_t = wbt[0:D, W1_C : W1_C + KF]
        w2_t = wbt[0:D, W2_C : W2_C + KF]
        wl_t = wbt[0:KF, WL_C : WL_C + OUT]
        b1_t = wft[0:KF, 0:1]
        b2_t = wft[0:KF, 1:2]
        bl_t = wft[0:OUT, 2:3]

        def eng(which):
            return {"A": nc.scalar, "D": nc.vector, "P": nc.gpsimd}[which]

        k = 0
        for st in range(n_super):
            s0 = st * super_
            hst = sb.tile([HS_P, super_], bf16, tag="hs")
            rbt = sb.tile([RB_P, super_], bf16, tag="rb")
            ot = sb.tile([OUT, super_], bf16, tag="ot")
            nc.sync.dma_start(out=hst[:], in_=hs[:, s0 : s0 + super_])
            nc.sync.dma_start(out=rbt[:], in_=rb[:, s0 : s0 + super_])
            for c in range(n_chunk):
                sl = slice(c * chunk, (c + 1) * chunk)
                ht = hst[0:D, sl]
                rt = rbt[0:KF, sl]
                a_ps = ps.tile([KF, chunk], f32, tag="a")
                g_ps = ps.tile([KF, chunk], f32, tag="g")
                o_ps = ps.tile([OUT, chunk], f32, tag="o")
                nc.tensor.matmul(out=a_ps[:], lhsT=w1_t, rhs=ht,
                                 start=True, stop=True)
                nc.tensor.matmul(out=g_ps[:], lhsT=w2_t, rhs=ht,
                                 start=True, stop=True)
                sig = vb.tile([KF, chunk], bf16, tag="sig")
                nc.scalar.activation(out=sig[:], in_=g_ps[:],
                                     func=mybir.ActivationFunctionType.Sigmoid,
                                     bias=b2_t, scale=1.0)
                glu = vb.tile([KF, chunk], bf16, tag="glu")
                se = eng(stst_eng[k % len(stst_eng)])
                se.scalar_tensor_tensor(out=glu[:], in0=a_ps[:],
                                        scalar=b1_t, in1=sig[:],
                                        op0=mybir.AluOpType.add,
                                        op1=mybir.AluOpType.mult)
                stk = vb.tile([KF, chunk], bf16, tag="stk")
                nc.vector.tensor_mul(out=stk[:], in0=glu[:], in1=rt)
                nc.tensor.matmul(out=o_ps[:], lhsT=wl_t, rhs=stk[:],
                                 start=True, stop=True)
                ce = copy_eng[k % len(copy_eng)]
                if ce == "A":
                    nc.scalar.activation(out=ot[:, sl], in_=o_ps[:],
                                         func=mybir.ActivationFunctionType.Identity,
                                         bias=bl_t, scale=1.0)
                else:
                    eng(ce).tensor_scalar_add(ot[:, sl], o_ps[:], bl_t)
                k += 1
            nc.sync.dma_start(out=outT[:, s0 : s0 + super_], in_=ot[:])
    nc.compile()
    return nc


def pack_weights(w1, b1, w2, b2, wl, bl):
    import ml_dtypes
    bf = ml_dtypes.bfloat16
    wbp = np.zeros((OUT, WB_F), dtype=bf)
    w1cat = np.asarray(w1, np.float32).transpose(1, 0, 2).reshape(D, KF)
    w2cat = np.asarray(w2, np.float32).transpose(1, 0, 2).reshape(D, KF)
    wbp[0:D, W1_C : W1_C + KF] = w1cat.astype(bf)
    wbp[0:D, W2_C : W2_C + KF] = w2cat.astype(bf)
    wbp[0:KF, WL_C : WL_C + OUT] = np.asarray(wl, np.float32).astype(bf)
    wfp = np.zeros((OUT, 3), dtype=np.float32)
    wfp[0:KF, 0] = np.asarray(b1, np.float32).reshape(KF)
    wfp[0:KF, 1] = np.asarray(b2, np.float32).reshape(KF)
    wfp[0:OUT, 2] = np.asarray(bl, np.float32).reshape(OUT)
    return wbp, wfp


def prep_inputs(rbf, h, idx_s, idx_t, w1, b1, w2, b2, wl, bl,
                e_total=E_TOTAL, n_cores=N_CORES):
    """Host-side marshaling: gather, transpose, replicate, bf16-cast, shard."""
    import ml_dtypes
    bf = ml_dtypes.bfloat16
    rbf = np.asarray(rbf, dtype=np.float32)
    h = np.asarray(h, dtype=np.float32)
    idx_s = np.asarray(idx_s).astype(np.int64)
    idx_t = np.asarray(idx_t).astype(np.int64)
    ec = e_total // n_cores

    hs = np.empty((HS_P, e_total), dtype=bf)
    hs[0:EMB, :] = h[idx_s].T.astype(bf)
    hs[EMB:D, :] = h[idx_t].T.astype(bf)
    rbfT = rbf.T.astype(bf)                      # [24, E]
    rb = np.empty((RB_P, e_total), dtype=bf)
    rb[0:24, :] = rbfT
    rb[24:48, :] = rbfT
    rb[48:72, :] = rbfT
    wbp, wfp = pack_weights(w1, b1, w2, b2, wl, bl)
    in_maps = []
    for i in range(n_cores):
        s = slice(i * ec, (i + 1) * ec)
        in_maps.append({"hs": np.ascontiguousarray(hs[:, s]),
                        "rb": np.ascontiguousarray(rb[:, s]),
                        "wb": wbp, "wf": wfp})
    return in_maps


def build_exec(nc, in_maps, donate=False):
    """Stage inputs on device once; return (fn, dev_args, outs).

    donate=False: fn(*dev_args, *outs) -> outputs (fresh buffers each call).
    donate=True:  fn(*dev_args, *outs) -> outputs; ping-pong the returned
    outputs back in as the next call's donated buffers."""
    import jax
    import jax.numpy as jnp
    from jax.sharding import Mesh, PartitionSpec, NamedSharding
    from jax.experimental.shard_map import shard_map
    import concourse.mybir as mybir
    from concourse.bass2jax import (_bass_exec_p, install_neuronx_cc_hook,
                                    partition_id_tensor)

    install_neuronx_cc_hook()
    n_cores = len(in_maps)
    in_names, out_names, out_avals = [], [], []
    partition_name = (nc.partition_id_tensor.name
                      if nc.partition_id_tensor else None)
    for alloc in nc.m.functions[0].allocations:
        if not isinstance(alloc, mybir.MemoryLocationSet):
            continue
        name = alloc.memorylocations[0].name
        if alloc.kind == "ExternalInput":
            if name != partition_name:
                in_names.append(name)
        elif alloc.kind == "ExternalOutput":
            out_names.append(name)
            out_avals.append(jax.core.ShapedArray(
                tuple(alloc.tensor_shape), mybir.dt.np(alloc.dtype)))
    n_params = len(in_names)
    all_in_names = list(in_names) + list(out_names)
    if partition_name is not None:
        all_in_names.append(partition_name)

    def _body(*args):
        operands = list(args)
        if partition_name is not None:
            operands.append(partition_id_tensor())
        return tuple(_bass_exec_p.bind(
            *operands,
            out_avals=tuple(out_avals),
            in_names=tuple(all_in_names),
            out_names=tuple(out_names),
            lowering_input_output_aliases=(),
            sim_require_finite=True,
            sim_require_nnan=True,
            nc=nc,
        ))

    devices = jax.devices()[:n_cores]
    mesh = Mesh(np.asarray(devices), ("core",))
    n_outs = len(out_names)
    in_specs = (PartitionSpec("core"),) * (n_params + n_outs)
    out_specs = (PartitionSpec("core"),) * n_outs
    donate_kw = {}
    if donate:
        donate_kw["donate_argnums"] = tuple(
            range(n_params, n_params + n_outs))
    fn = jax.jit(shard_map(_body, mesh=mesh, in_specs=in_specs,
                           out_specs=out_specs, check_rep=False),
                 keep_unused=True, **donate_kw)
    sh = NamedSharding(mesh, PartitionSpec("core"))
    dev_args = []
    for name in in_names:
        cat = np.concatenate([np.asarray(m[name]) for m in in_maps], axis=0)
        dev_args.append(jax.device_put(cat, sh))
    outs = []
    for av in out_avals:
        z = jnp.zeros((n_cores * av.shape[0], *av.shape[1:]), av.dtype)
        outs.append(jax.device_put(z, sh))
    return fn, dev_args, outs


def assemble_out(out_arrs, e_total=E_TOTAL, n_cores=N_CORES):
    """[8*128, ec] bf16 device output -> [E, 128] f32."""
    ec = e_total // n_cores
    out = np.empty((e_total, OUT), dtype=np.float32)
    a = np.asarray(out_arrs[0]).reshape(n_cores, OUT, ec)
    for i in range(n_cores):
        out[i * ec : (i + 1) * ec] = a[i].T.astype(np.float32)
    return out


def kernel(rbf, h, idx_s, idx_t, w1, b1, w2, b2, wl, bl):
    """Full-input entry point: shard across 8 cores, run the Bass kernel
    via run_bass_kernel_spmd, gather back to the full [E, 128] f32 output."""
    from concourse.bass_utils import run_bass_kernel_spmd

    e_total = rbf.shape[0]
    ec = e_total // N_CORES
    in_maps = prep_inputs(rbf, h, idx_s, idx_t, w1, b1, w2, b2, wl, bl,
                          e_total=e_total)
    nc = build_nc(ec)
    res = run_bass_kernel_spmd(nc, in_maps, list(range(N_CORES)))
    out = np.empty((e_total, OUT), dtype=np.float32)
    for i in range(N_CORES):
        out[i * ec : (i + 1) * ec] = \
            np.asarray(res.results[i]["outT"]).T.astype(np.float32)
    return out


# revision 6
# speedup vs baseline: 4.0339x; 3.0990x over previous
"""Trainium2 Bass kernel for nn_MEModule (gnn_message_passing).

Math per edge e (reference):
    h_emb = [h[idx_s[e]], h[idx_t[e]]]                 # [24]
    a     = h_emb @ w1cat + b1cat                      # [72]  (w1cat[d,(m,f)] = w1[m,d,f])
    g     = h_emb @ w2cat + b2cat                      # [72]
    glu   = a * sigmoid(g)                             # [72]
    stk   = glu * rbf3          (rbf3[(m,d)] = rbf[d]) # [72]
    out   = stk @ wl + bl                              # [128]

Device layout ("T-layout"): edges on the free dim, features on partitions.
Host pre-gathers h_emb -> hs [24, E] bf16 and pre-replicates rbf3 ->
rb [72, E] bf16 (separate tensors: SBUF operands spanning >32 partitions
must start at partition 0).  Output is produced as outT [128, E] bf16 and
de-transposed + f32-cast on the host.  8-way edge sharding; no collectives.

Per 500-edge chunk on device:
    a_ps = w1cat.T @ ht          (PE, bf16 inputs, PSUM f32 [72,500])
    g_ps = w2cat.T @ ht          (PE)
    sig  = sigmoid(g_ps + b2)    (ACT, bias AP, out SBUF bf16)
    glu  = (a_ps + b1) * sig     (DVE scalar_tensor_tensor)
    stk  = glu * rb_chunk        (DVE tensor_mul, all-SBUF bf16 -> 2x mode)
    o_ps = wl.T @ stk            (PE, PSUM [128,500])
    out  = o_ps + bl             (3 of 4 chunks: ACT Identity+bias;
                                  1 of 4: DVE tensor_scalar_add -> SBUF bf16)

Why this shape (evidence from the TimelineSim cost model, validated on HW):
  * fp32 matmuls cost 4 cycles/row on TRN2 -> bf16 operands are 4x faster
    on the PE; the v1 all-f32 kernel was PE-bound at 99% (1.68ms/core sim).
  * rbf3 host-replication kills v1's PE replication matmul and makes the
    stk multiply an all-SBUF 16-bit DVE op (2x mode).
  * bf16 stream + bf16 output halves DMA traffic (112MB/core).
  * PSUM->SBUF copies split 3:1 between ACT and DVE balances the two
    elementwise engines (GPSIMD cannot read PSUM, so Pool can't help).
  * Simulated 583us/core (v1: 1701us); engines DVE 86% / ACT 72% /
    PE 56% / DMA 55%.
HW (axon PJRT, 8 cores): rel err 6.1e-3 vs fp32 reference; steady-state
pipelined dispatch ~1.5-3.5ms/call (single-dispatch wall latency is
RTT-dominated at ~60-90ms on this tunnel).
"""

import numpy as np

N_CORES = 8
E_TOTAL = 2_000_000
EMB = 12
D = 24            # 2*EMB
KF = 72           # NUM_MODULES * D
HS_P = 24         # h_embT stream rows
RB_P = 72         # rbf3T stream rows
OUT = 128
SUPER = 5000      # edges per DMA supertile
CHUNK = 500       # edges per PSUM chunk (<=512 fp32 per bank)
STST_ENG = "D"    # glu op engine pattern (D=DVE)
COPY_ENG = "AAAD" # output-copy engine pattern (A=ACT, D=DVE)

# packed bf16 weight columns ([128, WB_F]); f32 biases ([128, 3])
W1_C, W2_C, WL_C = 0, 72, 144
WB_F = 272


def build_nc(e_shard: int, super_: int = SUPER, chunk: int = CHUNK,
             stst_eng: str = STST_ENG, copy_eng: str = COPY_ENG,
             repeat: int = 1):
    """repeat=K makes the NEFF run the full edge sweep K times back-to-back
    (identical output each pass).  Used by test.py to measure per-execution
    HW time with the per-dispatch RPC overhead amortized over K."""
    from contextlib import ExitStack

    import concourse.tile as tile
    from concourse import bacc, mybir

    f32 = mybir.dt.float32
    bf16 = mybir.dt.bfloat16
    assert e_shard % super_ == 0 and super_ % chunk == 0
    n_super = e_shard // super_
    n_chunk = super_ // chunk

    try:
        from concourse._compat import get_trn_type
        trn = get_trn_type() or "TRN2"
    except Exception:
        trn = "TRN2"
    nc = bacc.Bacc(trn, target_bir_lowering=False, debug=False)
    hs = nc.declare_dram_parameter("hs", [HS_P, e_shard], bf16, isOutput=False)
    rb = nc.declare_dram_parameter("rb", [RB_P, e_shard], bf16, isOutput=False)
    wb = nc.declare_dram_parameter("wb", [OUT, WB_F], bf16, isOutput=False)
    wf = nc.declare_dram_parameter("wf", [OUT, 3], f32, isOutput=False)
    outT = nc.declare_dram_parameter("outT", [OUT, e_shard], bf16, isOutput=True)

    with ExitStack() as ctx:
        tc = ctx.enter_context(tile.TileContext(nc))
        wpool = ctx.enter_context(tc.tile_pool(name="weights", bufs=1))
        sb = ctx.enter_context(tc.tile_pool(name="sbuf", bufs=2))
        vb = ctx.enter_context(tc.tile_pool(name="vecbuf", bufs=2))
        ps = ctx.enter_context(tc.tile_pool(name="psum", bufs=2, space="PSUM"))

        wbt = wpool.tile([OUT, WB_F], bf16, tag="wb")
        wft = wpool.tile([OUT, 3], f32, tag="wf")
        nc.sync.dma_start(out=wbt[:], in_=wb[:])
        nc.sync.dma_start(out=wft[:], in_=wf[:])
        w1_t = wbt[0:D, W1_C : W1_C + KF]
        w2_t = wbt[0:D, W2_C : W2_C + KF]
        wl_t = wbt[0:KF, WL_C : WL_C + OUT]
        b1_t = wft[0:KF, 0:1]
        b2_t = wft[0:KF, 1:2]
        bl_t = wft[0:OUT, 2:3]

        def eng(which):
            return {"A": nc.scalar, "D": nc.vector, "P": nc.gpsimd}[which]

        k = 0
        for rep in range(repeat):
          for st in range(n_super):
            s0 = st * super_
            hst = sb.tile([HS_P, super_], bf16, tag="hs")
            rbt = sb.tile([RB_P, super_], bf16, tag="rb")
            ot = sb.tile([OUT, super_], bf16, tag="ot")
            nc.sync.dma_start(out=hst[:], in_=hs[:, s0 : s0 + super_])
            nc.sync.dma_start(out=rbt[:], in_=rb[:, s0 : s0 + super_])
            for c in range(n_chunk):
                sl = slice(c * chunk, (c + 1) * chunk)
                ht = hst[0:D, sl]
                rt = rbt[0:KF, sl]
                a_ps = ps.tile([KF, chunk], f32, tag="a")
                g_ps = ps.tile([KF, chunk], f32, tag="g")
                o_ps = ps.tile([OUT, chunk], f32, tag="o")
                nc.tensor.matmul(out=a_ps[:], lhsT=w1_t, rhs=ht,
                                 start=True, stop=True)
                nc.tensor.matmul(out=g_ps[:], lhsT=w2_t, rhs=ht,
                                 start=True, stop=True)
                sig = vb.tile([KF, chunk], bf16, tag="sig")
                nc.scalar.activation(out=sig[:], in_=g_ps[:],
                                     func=mybir.ActivationFunctionType.Sigmoid,
                                     bias=b2_t, scale=1.0)
                glu = vb.tile([KF, chunk], bf16, tag="glu")
                se = eng(stst_eng[k % len(stst_eng)])
                se.scalar_tensor_tensor(out=glu[:], in0=a_ps[:],
                                        scalar=b1_t, in1=sig[:],
                                        op0=mybir.AluOpType.add,
                                        op1=mybir.AluOpType.mult)
                stk = vb.tile([KF, chunk], bf16, tag="stk")
                nc.vector.tensor_mul(out=stk[:], in0=glu[:], in1=rt)
                nc.tensor.matmul(out=o_ps[:], lhsT=wl_t, rhs=stk[:],
                                 start=True, stop=True)
                ce = copy_eng[k % len(copy_eng)]
                if ce == "A":
                    nc.scalar.activation(out=ot[:, sl], in_=o_ps[:],
                                         func=mybir.ActivationFunctionType.Identity,
                                         bias=bl_t, scale=1.0)
                else:
                    eng(ce).tensor_scalar_add(ot[:, sl], o_ps[:], bl_t)
                k += 1
            nc.sync.dma_start(out=outT[:, s0 : s0 + super_], in_=ot[:])
    nc.compile()
    return nc


def pack_weights(w1, b1, w2, b2, wl, bl):
    import ml_dtypes
    bf = ml_dtypes.bfloat16
    wbp = np.zeros((OUT, WB_F), dtype=bf)
    w1cat = np.asarray(w1, np.float32).transpose(1, 0, 2).reshape(D, KF)
    w2cat = np.asarray(w2, np.float32).transpose(1, 0, 2).reshape(D, KF)
    wbp[0:D, W1_C : W1_C + KF] = w1cat.astype(bf)
    wbp[0:D, W2_C : W2_C + KF] = w2cat.astype(bf)
    wbp[0:KF, WL_C : WL_C + OUT] = np.asarray(wl, np.float32).astype(bf)
    wfp = np.zeros((OUT, 3), dtype=np.float32)
    wfp[0:KF, 0] = np.asarray(b1, np.float32).reshape(KF)
    wfp[0:KF, 1] = np.asarray(b2, np.float32).reshape(KF)
    wfp[0:OUT, 2] = np.asarray(bl, np.float32).reshape(OUT)
    return wbp, wfp


def prep_inputs(rbf, h, idx_s, idx_t, w1, b1, w2, b2, wl, bl,
                e_total=E_TOTAL, n_cores=N_CORES):
    """Host-side marshaling: gather, transpose, replicate, bf16-cast, shard."""
    import ml_dtypes
    bf = ml_dtypes.bfloat16
    rbf = np.asarray(rbf, dtype=np.float32)
    h = np.asarray(h, dtype=np.float32)
    idx_s = np.asarray(idx_s).astype(np.int64)
    idx_t = np.asarray(idx_t).astype(np.int64)
    ec = e_total // n_cores

    hs = np.empty((HS_P, e_total), dtype=bf)
    hs[0:EMB, :] = h[idx_s].T.astype(bf)
    hs[EMB:D, :] = h[idx_t].T.astype(bf)
    rbfT = rbf.T.astype(bf)                      # [24, E]
    rb = np.empty((RB_P, e_total), dtype=bf)
    rb[0:24, :] = rbfT
    rb[24:48, :] = rbfT
    rb[48:72, :] = rbfT
    wbp, wfp = pack_weights(w1, b1, w2, b2, wl, bl)
    in_maps = []
    for i in range(n_cores):
        s = slice(i * ec, (i + 1) * ec)
        in_maps.append({"hs": np.ascontiguousarray(hs[:, s]),
                        "rb": np.ascontiguousarray(rb[:, s]),
                        "wb": wbp, "wf": wfp})
    return in_maps


def build_exec(nc, in_maps, donate=False):
    """Stage inputs on device once; return (fn, dev_args, outs).

    donate=False: fn(*dev_args, *outs) -> outputs (fresh buffers each call).
    donate=True:  fn(*dev_args, *outs) -> outputs; ping-pong the returned
    outputs back in as the next call's donated buffers."""
    import jax
    import jax.numpy as jnp
    from jax.sharding import Mesh, PartitionSpec, NamedSharding
    from jax.experimental.shard_map import shard_map
    import concourse.mybir as mybir
    from concourse.bass2jax import (_bass_exec_p, install_neuronx_cc_hook,
                                    partition_id_tensor)

    install_neuronx_cc_hook()
    n_cores = len(in_maps)
    in_names, out_names, out_avals = [], [], []
    partition_name = (nc.partition_id_tensor.name
                      if nc.partition_id_tensor else None)
    for alloc in nc.m.functions[0].allocations:
        if not isinstance(alloc, mybir.MemoryLocationSet):
            continue
        name = alloc.memorylocations[0].name
        if alloc.kind == "ExternalInput":
            if name != partition_name:
                in_names.append(name)
        elif alloc.kind == "ExternalOutput":
            out_names.append(name)
            out_avals.append(jax.core.ShapedArray(
                tuple(alloc.tensor_shape), mybir.dt.np(alloc.dtype)))
    n_params = len(in_names)
    all_in_names = list(in_names) + list(out_names)
    if partition_name is not None:
        all_in_names.append(partition_name)

    def _body(*args):
        operands = list(args)
        if partition_name is not None:
            operands.append(partition_id_tensor())
        return tuple(_bass_exec_p.bind(
            *operands,
            out_avals=tuple(out_avals),
            in_names=tuple(all_in_names),
            out_names=tuple(out_names),
            lowering_input_output_aliases=(),
            sim_require_finite=True,
            sim_require_nnan=True,
            nc=nc,
        ))

    devices = jax.devices()[:n_cores]
    mesh = Mesh(np.asarray(devices), ("core",))
    n_outs = len(out_names)
    in_specs = (PartitionSpec("core"),) * (n_params + n_outs)
    out_specs = (PartitionSpec("core"),) * n_outs
    donate_kw = {}
    if donate:
        donate_kw["donate_argnums"] = tuple(
            range(n_params, n_params + n_outs))
    fn = jax.jit(shard_map(_body, mesh=mesh, in_specs=in_specs,
                           out_specs=out_specs, check_rep=False),
                 keep_unused=True, **donate_kw)
    sh = NamedSharding(mesh, PartitionSpec("core"))
    dev_args = []
    for name in in_names:
        cat = np.concatenate([np.asarray(m[name]) for m in in_maps], axis=0)
        dev_args.append(jax.device_put(cat, sh))
    outs = []
    for av in out_avals:
        z = jnp.zeros((n_cores * av.shape[0], *av.shape[1:]), av.dtype)
        outs.append(jax.device_put(z, sh))
    return fn, dev_args, outs


def assemble_out(out_arrs, e_total=E_TOTAL, n_cores=N_CORES):
    """[8*128, ec] bf16 device output -> [E, 128] f32."""
    ec = e_total // n_cores
    out = np.empty((e_total, OUT), dtype=np.float32)
    a = np.asarray(out_arrs[0]).reshape(n_cores, OUT, ec)
    for i in range(n_cores):
        out[i * ec : (i + 1) * ec] = a[i].T.astype(np.float32)
    return out


def kernel(rbf, h, idx_s, idx_t, w1, b1, w2, b2, wl, bl):
    """Full-input entry point: shard across 8 cores, run the Bass kernel
    via run_bass_kernel_spmd, gather back to the full [E, 128] f32 output."""
    from concourse.bass_utils import run_bass_kernel_spmd

    e_total = rbf.shape[0]
    ec = e_total // N_CORES
    in_maps = prep_inputs(rbf, h, idx_s, idx_t, w1, b1, w2, b2, wl, bl,
                          e_total=e_total)
    nc = build_nc(ec)
    res = run_bass_kernel_spmd(nc, in_maps, list(range(N_CORES)))
    out = np.empty((e_total, OUT), dtype=np.float32)
    for i in range(N_CORES):
        out[i * ec : (i + 1) * ec] = \
            np.asarray(res.results[i]["outT"]).T.astype(np.float32)
    return out


# revision 7
# speedup vs baseline: 4.5047x; 1.1167x over previous
"""Trainium2 Bass kernel for nn_MEModule (gnn_message_passing).

Math per edge e (reference):
    h_emb = [h[idx_s[e]], h[idx_t[e]]]                 # [24]
    a     = h_emb @ w1cat + b1cat                      # [72]  (w1cat[d,(m,f)] = w1[m,d,f])
    g     = h_emb @ w2cat + b2cat                      # [72]
    glu   = a * sigmoid(g)                             # [72]
    stk   = glu * rbf3          (rbf3[(m,d)] = rbf[d]) # [72]
    out   = stk @ wl + bl                              # [128]

Device layout ("T-layout"): edges on the free dim, features on partitions.
Host pre-gathers h_emb -> hs [24, E] bf16 and pre-replicates rbf3 ->
rb [72, E] bf16 (separate tensors: SBUF operands spanning >32 partitions
must start at partition 0).  Output is produced as outT [128, E] bf16 and
de-transposed + f32-cast on the host.  8-way edge sharding; no collectives.

Per 500-edge chunk on device:
    a_ps = w1cat.T @ ht          (PE, bf16 inputs, PSUM f32 [72,500])
    g_ps = w2cat.T @ ht          (PE)
    sig  = sigmoid(g_ps + b2)    (ACT, bias AP, out SBUF bf16)
    glu  = (a_ps + b1) * sig     (DVE scalar_tensor_tensor)
    stk  = glu * rb_chunk        (DVE tensor_mul, all-SBUF bf16 -> 2x mode)
    o_ps = wl.T @ stk            (PE, PSUM [128,500])
    out  = o_ps + bl             (3 of 4 chunks: ACT Identity+bias;
                                  1 of 4: DVE tensor_scalar_add -> SBUF bf16)

Why this shape (evidence from the TimelineSim cost model, validated on HW):
  * fp32 matmuls cost 4 cycles/row on TRN2 -> bf16 operands are 4x faster
    on the PE; the v1 all-f32 kernel was PE-bound at 99% (1.68ms/core sim).
  * rbf3 host-replication kills v1's PE replication matmul and makes the
    stk multiply an all-SBUF 16-bit DVE op (2x mode).
  * bf16 stream + bf16 output halves DMA traffic (112MB/core).
  * PSUM->SBUF copies split 3:1 between ACT and DVE balances the two
    elementwise engines (GPSIMD cannot read PSUM, so Pool can't help).
  * Simulated 583us/core (v1: 1701us); engines DVE 86% / ACT 72% /
    PE 56% / DMA 55%.
HW (axon PJRT, 8 cores): rel err 6.1e-3 vs fp32 reference; steady-state
pipelined dispatch ~1.5-3.5ms/call (single-dispatch wall latency is
RTT-dominated at ~60-90ms on this tunnel).
"""

import numpy as np

N_CORES = 8
E_TOTAL = 2_000_000
EMB = 12
D = 24            # 2*EMB
KF = 72           # NUM_MODULES * D
HS_P = 24         # h_embT stream rows
RB_P = 72         # rbf3T stream rows
OUT = 128
SUPER = 5000      # edges per DMA supertile
CHUNK = 500       # edges per PSUM chunk (<=512 fp32 per bank)
STST_ENG = "D"    # glu op engine pattern (D=DVE)
COPY_ENG = "AAAD" # output-copy engine pattern (A=ACT, D=DVE)

# packed bf16 weight columns ([128, WB_F]); f32 biases ([128, 3])
W1_C, W2_C, WL_C = 0, 72, 144
WB_F = 272


def build_nc(e_shard: int, super_: int = SUPER, chunk: int = CHUNK,
             stst_eng: str = STST_ENG, copy_eng: str = COPY_ENG,
             mul_eng: str = "D", vb_bufs: int = 2, repeat: int = 1):
    """repeat=K makes the NEFF run the full edge sweep K times back-to-back
    (identical output each pass).  Used by test.py to measure per-execution
    HW time with the per-dispatch RPC overhead amortized over K."""
    from contextlib import ExitStack

    import concourse.tile as tile
    from concourse import bacc, mybir

    f32 = mybir.dt.float32
    bf16 = mybir.dt.bfloat16
    assert e_shard % super_ == 0 and super_ % chunk == 0
    n_super = e_shard // super_
    n_chunk = super_ // chunk

    try:
        from concourse._compat import get_trn_type
        trn = get_trn_type() or "TRN2"
    except Exception:
        trn = "TRN2"
    nc = bacc.Bacc(trn, target_bir_lowering=False, debug=False)
    hs = nc.declare_dram_parameter("hs", [HS_P, e_shard], bf16, isOutput=False)
    rb = nc.declare_dram_parameter("rb", [RB_P, e_shard], bf16, isOutput=False)
    wb = nc.declare_dram_parameter("wb", [OUT, WB_F], bf16, isOutput=False)
    wf = nc.declare_dram_parameter("wf", [OUT, 3], f32, isOutput=False)
    outT = nc.declare_dram_parameter("outT", [OUT, e_shard], bf16, isOutput=True)

    with ExitStack() as ctx:
        tc = ctx.enter_context(tile.TileContext(nc))
        wpool = ctx.enter_context(tc.tile_pool(name="weights", bufs=1))
        sb = ctx.enter_context(tc.tile_pool(name="sbuf", bufs=2))
        vb = ctx.enter_context(tc.tile_pool(name="vecbuf", bufs=vb_bufs))
        ps = ctx.enter_context(tc.tile_pool(name="psum", bufs=2, space="PSUM"))

        wbt = wpool.tile([OUT, WB_F], bf16, tag="wb")
        wft = wpool.tile([OUT, 3], f32, tag="wf")
        nc.sync.dma_start(out=wbt[:], in_=wb[:])
        nc.sync.dma_start(out=wft[:], in_=wf[:])
        w1_t = wbt[0:D, W1_C : W1_C + KF]
        w2_t = wbt[0:D, W2_C : W2_C + KF]
        wl_t = wbt[0:KF, WL_C : WL_C + OUT]
        b1_t = wft[0:KF, 0:1]
        b2_t = wft[0:KF, 1:2]
        bl_t = wft[0:OUT, 2:3]

        def eng(which):
            return {"A": nc.scalar, "D": nc.vector, "P": nc.gpsimd}[which]

        k = 0
        for rep in range(repeat):
          for st in range(n_super):
            s0 = st * super_
            hst = sb.tile([HS_P, super_], bf16, tag="hs")
            rbt = sb.tile([RB_P, super_], bf16, tag="rb")
            ot = sb.tile([OUT, super_], bf16, tag="ot")
            nc.sync.dma_start(out=hst[:], in_=hs[:, s0 : s0 + super_])
            nc.sync.dma_start(out=rbt[:], in_=rb[:, s0 : s0 + super_])
            for c in range(n_chunk):
                sl = slice(c * chunk, (c + 1) * chunk)
                ht = hst[0:D, sl]
                rt = rbt[0:KF, sl]
                a_ps = ps.tile([KF, chunk], f32, tag="a")
                g_ps = ps.tile([KF, chunk], f32, tag="g")
                o_ps = ps.tile([OUT, chunk], f32, tag="o")
                nc.tensor.matmul(out=a_ps[:], lhsT=w1_t, rhs=ht,
                                 start=True, stop=True)
                nc.tensor.matmul(out=g_ps[:], lhsT=w2_t, rhs=ht,
                                 start=True, stop=True)
                sig = vb.tile([KF, chunk], bf16, tag="sig")
                nc.scalar.activation(out=sig[:], in_=g_ps[:],
                                     func=mybir.ActivationFunctionType.Sigmoid,
                                     bias=b2_t, scale=1.0)
                glu = vb.tile([KF, chunk], bf16, tag="glu")
                se = eng(stst_eng[k % len(stst_eng)])
                se.scalar_tensor_tensor(out=glu[:], in0=a_ps[:],
                                        scalar=b1_t, in1=sig[:],
                                        op0=mybir.AluOpType.add,
                                        op1=mybir.AluOpType.mult)
                stk = vb.tile([KF, chunk], bf16, tag="stk")
                eng(mul_eng[k % len(mul_eng)]).tensor_mul(
                    out=stk[:], in0=glu[:], in1=rt)
                nc.tensor.matmul(out=o_ps[:], lhsT=wl_t, rhs=stk[:],
                                 start=True, stop=True)
                ce = copy_eng[k % len(copy_eng)]
                if ce == "A":
                    nc.scalar.activation(out=ot[:, sl], in_=o_ps[:],
                                         func=mybir.ActivationFunctionType.Identity,
                                         bias=bl_t, scale=1.0)
                else:
                    eng(ce).tensor_scalar_add(ot[:, sl], o_ps[:], bl_t)
                k += 1
            nc.sync.dma_start(out=outT[:, s0 : s0 + super_], in_=ot[:])
    nc.compile()
    return nc


def pack_weights(w1, b1, w2, b2, wl, bl):
    import ml_dtypes
    bf = ml_dtypes.bfloat16
    wbp = np.zeros((OUT, WB_F), dtype=bf)
    w1cat = np.asarray(w1, np.float32).transpose(1, 0, 2).reshape(D, KF)
    w2cat = np.asarray(w2, np.float32).transpose(1, 0, 2).reshape(D, KF)
    wbp[0:D, W1_C : W1_C + KF] = w1cat.astype(bf)
    wbp[0:D, W2_C : W2_C + KF] = w2cat.astype(bf)
    wbp[0:KF, WL_C : WL_C + OUT] = np.asarray(wl, np.float32).astype(bf)
    wfp = np.zeros((OUT, 3), dtype=np.float32)
    wfp[0:KF, 0] = np.asarray(b1, np.float32).reshape(KF)
    wfp[0:KF, 1] = np.asarray(b2, np.float32).reshape(KF)
    wfp[0:OUT, 2] = np.asarray(bl, np.float32).reshape(OUT)
    return wbp, wfp


def prep_inputs(rbf, h, idx_s, idx_t, w1, b1, w2, b2, wl, bl,
                e_total=E_TOTAL, n_cores=N_CORES):
    """Host-side marshaling: gather, transpose, replicate, bf16-cast, shard."""
    import ml_dtypes
    bf = ml_dtypes.bfloat16
    rbf = np.asarray(rbf, dtype=np.float32)
    h = np.asarray(h, dtype=np.float32)
    idx_s = np.asarray(idx_s).astype(np.int64)
    idx_t = np.asarray(idx_t).astype(np.int64)
    ec = e_total // n_cores

    hs = np.empty((HS_P, e_total), dtype=bf)
    hs[0:EMB, :] = h[idx_s].T.astype(bf)
    hs[EMB:D, :] = h[idx_t].T.astype(bf)
    rbfT = rbf.T.astype(bf)                      # [24, E]
    rb = np.empty((RB_P, e_total), dtype=bf)
    rb[0:24, :] = rbfT
    rb[24:48, :] = rbfT
    rb[48:72, :] = rbfT
    wbp, wfp = pack_weights(w1, b1, w2, b2, wl, bl)
    in_maps = []
    for i in range(n_cores):
        s = slice(i * ec, (i + 1) * ec)
        in_maps.append({"hs": np.ascontiguousarray(hs[:, s]),
                        "rb": np.ascontiguousarray(rb[:, s]),
                        "wb": wbp, "wf": wfp})
    return in_maps


def build_exec(nc, in_maps, donate=False):
    """Stage inputs on device once; return (fn, dev_args, outs).

    donate=False: fn(*dev_args, *outs) -> outputs (fresh buffers each call).
    donate=True:  fn(*dev_args, *outs) -> outputs; ping-pong the returned
    outputs back in as the next call's donated buffers."""
    import jax
    import jax.numpy as jnp
    from jax.sharding import Mesh, PartitionSpec, NamedSharding
    from jax.experimental.shard_map import shard_map
    import concourse.mybir as mybir
    from concourse.bass2jax import (_bass_exec_p, install_neuronx_cc_hook,
                                    partition_id_tensor)

    install_neuronx_cc_hook()
    n_cores = len(in_maps)
    in_names, out_names, out_avals = [], [], []
    partition_name = (nc.partition_id_tensor.name
                      if nc.partition_id_tensor else None)
    for alloc in nc.m.functions[0].allocations:
        if not isinstance(alloc, mybir.MemoryLocationSet):
            continue
        name = alloc.memorylocations[0].name
        if alloc.kind == "ExternalInput":
            if name != partition_name:
                in_names.append(name)
        elif alloc.kind == "ExternalOutput":
            out_names.append(name)
            out_avals.append(jax.core.ShapedArray(
                tuple(alloc.tensor_shape), mybir.dt.np(alloc.dtype)))
    n_params = len(in_names)
    all_in_names = list(in_names) + list(out_names)
    if partition_name is not None:
        all_in_names.append(partition_name)

    def _body(*args):
        operands = list(args)
        if partition_name is not None:
            operands.append(partition_id_tensor())
        return tuple(_bass_exec_p.bind(
            *operands,
            out_avals=tuple(out_avals),
            in_names=tuple(all_in_names),
            out_names=tuple(out_names),
            lowering_input_output_aliases=(),
            sim_require_finite=True,
            sim_require_nnan=True,
            nc=nc,
        ))

    devices = jax.devices()[:n_cores]
    mesh = Mesh(np.asarray(devices), ("core",))
    n_outs = len(out_names)
    in_specs = (PartitionSpec("core"),) * (n_params + n_outs)
    out_specs = (PartitionSpec("core"),) * n_outs
    donate_kw = {}
    if donate:
        donate_kw["donate_argnums"] = tuple(
            range(n_params, n_params + n_outs))
    fn = jax.jit(shard_map(_body, mesh=mesh, in_specs=in_specs,
                           out_specs=out_specs, check_rep=False),
                 keep_unused=True, **donate_kw)
    sh = NamedSharding(mesh, PartitionSpec("core"))
    dev_args = []
    for name in in_names:
        cat = np.concatenate([np.asarray(m[name]) for m in in_maps], axis=0)
        dev_args.append(jax.device_put(cat, sh))
    outs = []
    for av in out_avals:
        z = jnp.zeros((n_cores * av.shape[0], *av.shape[1:]), av.dtype)
        outs.append(jax.device_put(z, sh))
    return fn, dev_args, outs


def assemble_out(out_arrs, e_total=E_TOTAL, n_cores=N_CORES):
    """[8*128, ec] bf16 device output -> [E, 128] f32."""
    ec = e_total // n_cores
    out = np.empty((e_total, OUT), dtype=np.float32)
    a = np.asarray(out_arrs[0]).reshape(n_cores, OUT, ec)
    for i in range(n_cores):
        out[i * ec : (i + 1) * ec] = a[i].T.astype(np.float32)
    return out


def kernel(rbf, h, idx_s, idx_t, w1, b1, w2, b2, wl, bl):
    """Full-input entry point: shard across 8 cores, run the Bass kernel
    via run_bass_kernel_spmd, gather back to the full [E, 128] f32 output."""
    from concourse.bass_utils import run_bass_kernel_spmd

    e_total = rbf.shape[0]
    ec = e_total // N_CORES
    in_maps = prep_inputs(rbf, h, idx_s, idx_t, w1, b1, w2, b2, wl, bl,
                          e_total=e_total)
    nc = build_nc(ec)
    res = run_bass_kernel_spmd(nc, in_maps, list(range(N_CORES)))
    out = np.empty((e_total, OUT), dtype=np.float32)
    for i in range(N_CORES):
        out[i * ec : (i + 1) * ec] = \
            np.asarray(res.results[i]["outT"]).T.astype(np.float32)
    return out
